# revision 32
# baseline (speedup 1.0000x reference)
"""Trainium2 Bass kernel: memory-slot cross-attention (nn_LocalConstructorMulti).

Algebraic restructuring vs the reference:
    scores[b,h,n,s] = (Q[n,h,:] . K[b,s,h,:]) / 8
                    = hs[b,s,:] . qe[h*8+n,:]        with qe = fold(Wk, Wq, ms)
    out[b,n,h,:]    = Wv_h @ ctx[b,h*8+n,:]          with ctx = attn-weighted
                                                     sum of hidden states
    y[b,n,:]        = sum_h Wo_h (Wv_h ctx_hn)

So the device only computes, per (batch, seq-half) core:
    Phase 1: s[hn, s]  = qe8.T @ hsT8   (fp8, DoubleRow)   [64, cap]
             p~[hn, s] = exp(s * EXPSCALE + maskbias)      (unnormalized)
    Phase 2: dev[hn,:] = (p~ - 1) @ hs8 (fp8, DoubleRow)   [64, 4096]
             den[hn]   = sum_s p~                          (ACT accum_out)
with ctx = S + dev assembled on host (S = exact f32 token sum). The tiny
Wv/Wo projections and the 1/den normalization also happen on host (linear
ops commute with the attention sum; den differs per (h,n) so normalization
must precede the Wo mix, after summing the two seq-halves).

This removes the big K/V projections entirely and the kernel becomes
DMA-bound on reading hs once in fp8e4m3 s-major plus once transposed
(fp8 is accurate enough for scores because quantization noise averages
over the 4096-long contraction, and for ctx because only the small
fluctuation q = p~ - 1 multiplies it — see the hs8 declaration comment).

Masked tokens contribute exactly zero (p~ = exp(-1e9 * EXPSCALE) = 0),
so the host drops them before sharding: only unmasked tokens are shipped,
padded to a per-core capacity rounded up to a multiple of 32 (1056 for
this mask; the trailing cap % 128 rows run as a ragged partial-width
tile). Padding rows get hs = 0 and bias -1e9, i.e. they also contribute
exactly zero.

Sharding: 8 cores = 4 batches x 2 halves of each batch's unmasked-token
list. Host sums the halves' unnormalized ctx/den, normalizes, applies
Wv/Wo.
"""

import sys

if "/opt/trn_rl_repo" not in sys.path:
    sys.path.insert(0, "/opt/trn_rl_repo")

import ml_dtypes
import numpy as np

import concourse.bass as bass  # noqa: F401  (AP helpers)
import concourse.mybir as mybir
import concourse.tile as tile
from concourse import bacc
from concourse.bass_utils import run_bass_kernel_spmd
from concourse.masks import make_identity

BF16 = mybir.dt.bfloat16
F32 = mybir.dt.float32
F8 = mybir.dt.float8e4
npbf16 = ml_dtypes.bfloat16
npf8 = ml_dtypes.float8_e4m3

B, S, HID = 4, 4096, 4096
SLOTS, HEADS, BD = 8, 8, 512
HD = BD // HEADS  # 64
N_CORES = 8
HALVES = 2
HN = HEADS * SLOTS  # 64 (head, slot) pairs
NK = HID // 128  # 32 contraction k-tiles
NCG = HID // 512  # 8 ctx col groups (PSUM banks)
QSCALE = 512.0  # qe pre-scale so fp8 values sit in the normal range
SCALE = 1.0 / float(np.sqrt(HD))
EXPSCALE = 1.0 / (QSCALE / SCALE)  # undo QSCALE, apply 1/sqrt(HD)
MASK_NEG = -1.0e9  # after *EXPSCALE -> -244k -> exp -> 0
# Per-core token capacity (multiple of 32): roundup(max tokens per core, 32)
# computed at prep time; DEF_CAP only sizes the default module for warmup.
# The trailing cap%128 rows form a ragged tile handled at partial width, so
# padding waste is at most 31 tokens (vs 128-aligned capacities).
DEF_CAP = 1056
# No-op filler matmuls (zero stationary, +0 accumulate) appended to each
# ctx tile burst: they keep the Tensor engine continuously busy between hs
# tile arrivals so it holds its full 2.4 GHz p-state (any idle gap drops
# it to 1.2 GHz and the real matmuls take 2x longer).
PE_FILL = 0

# test.py can flip this to capture an NTFF profile; harness never touches it.
TRACE = False
TRACE_CORES = None
LAST_RESULT = None

_cache = {}


def _chunks(cap):
    """Score chunk widths: 512s then an optional remainder (PSUM bank = 512)."""
    out = [512] * (cap // 512)
    if cap % 512:
        out.append(cap % 512)
    assert sum(out) == cap
    return out


def _round_cap(need):
    """Token capacity for a per-core token count: multiple of 32, >= 256."""
    return max(256, ((need + 31) // 32) * 32)


def _build_module(cap, reps=1):
    """Emit + compile the single-core Bass module (same NEFF on all cores).

    reps > 1 emits the identical kernel body `reps` times back-to-back in
    one NEFF (each rep redoes every DMA and matmul, writing the same
    outputs with identical data). Used only for timing: the marginal cost
    per rep measures true device execution throughput with the multi-ms
    axon per-execute RPC overhead amortized away.
    """
    assert cap % 32 == 0
    full = cap // 128  # full 128-row token tiles
    rag = cap % 128  # trailing ragged tile rows (multiple of 32)
    ntile = full + (1 if rag else 0)
    widths = _chunks(cap)
    nch = len(widths)

    nc = bacc.Bacc("TRN2", target_bir_lowering=False, debug=False, num_devices=N_CORES)

    # hsT8 is pre-tiled on host: chunk ch occupies cols [NK*off, NK*(off+w))
    # with per-partition-contiguous [ko, col] blocks, so each chunk DMA is a
    # single long run per partition (short runs pay a 2x DMA penalty).
    hsT8 = nc.dram_tensor("hsT8", [128, NK * cap], F8, kind="ExternalInput").ap()
    # ctx phase reads fp8 hidden states and contracts q = p~ - 1 against
    # them: attention here is near-uniform (p~ in [0.91, 1.10]), so
    # sum_s p~*hs = S + sum_s q*hs8 + eps, where S (the exact token sum) is
    # added on HOST in f32 and eps (~0.06% RMS) collects the q-fp8 and
    # hs-fp8 quantization cross terms. With q in fp8 too, both ctx matmul
    # operands are fp8 -> DoubleRow contracts two 128-token tiles per
    # matmul at 0.5 cycles/row.
    hs8 = nc.dram_tensor("hs8", [cap, HID], F8, kind="ExternalInput").ap()
    qe8 = nc.dram_tensor("qe8", [128, NK * HN], F8, kind="ExternalInput").ap()
    mbT = nc.dram_tensor("mbT", [1, cap], BF16, kind="ExternalInput").ap()
    ctx_out = nc.dram_tensor("ctx_out", [HN, HID], BF16, kind="ExternalOutput").ap()
    den_out = nc.dram_tensor("den_out", [HN, 1], F32, kind="ExternalOutput").ap()

    with tile.TileContext(nc) as tc:
      for _rep in range(reps):
        with (
            tc.tile_pool(name="consts", bufs=1) as consts,
            tc.tile_pool(name="hsp", bufs=9) as hsp,
        ):
            # ---- resident tensors -----------------------------------------
            # DMA ring split: sync (SP) carries qe/mb + the hsT chunks
            # (they gate the long phase-1 chain), scalar (ACT) streams the
            # phase-2 hs8 tiles concurrently — on HW the two HWDGE rings
            # together sustain more bandwidth than one (the cost-model sim
            # says otherwise; believe the hardware). den_out's DMA is
            # emitted at the very END: it waits on all of phase 1, and
            # sitting between the hsT and hs8 DMAs on one ring it stalled
            # phase 2's stream (~4 us).
            hsT_t = [consts.tile([128, NK, w], F8, name=f"hsT{ch}")
                     for ch, w in enumerate(widths)]
            qe_sb = consts.tile([128, NK, HN], F8)
            mb_sb = consts.tile([1, cap], BF16)
            nc.sync.dma_start(
                out=qe_sb, in_=qe8.rearrange("p (a b) -> p a b", a=NK)
            )
            nc.sync.dma_start(out=mb_sb, in_=mbT)
            off = 0
            for ch, w in enumerate(widths):
                nc.sync.dma_start(
                    out=hsT_t[ch],
                    in_=hsT8[:, NK * off : NK * (off + w)].rearrange(
                        "p (a b) -> p a b", a=NK
                    ),
                )
                off += w
            # phase-2 hs row-tiles: one resident [128, full, HID] tile
            # streamed by TWO merged DMAs (plus the ragged tail) on the
            # scalar ring. Descriptor efficiency is identical to per-tile
            # DMAs (4 KB runs per partition either way), but the DGE issue
            # cost on the ring head drops from ~10x667 ns to ~3x667 ns.
            npair = full // 2
            hs_big = consts.tile([128, full, HID], F8, name="hsbig")
            h1 = (full + 1) // 2
            nc.scalar.dma_start(
                out=hs_big[:, 0:h1, :],
                in_=hs8[0 : h1 * 128, :].rearrange("(a p) f -> p a f", p=128),
            )
            if full > h1:
                nc.scalar.dma_start(
                    out=hs_big[:, h1:full, :],
                    in_=hs8[h1 * 128 : full * 128, :].rearrange(
                        "(a p) f -> p a f", p=128
                    ),
                )
            hs_pair = [hs_big[:, 2 * tp : 2 * tp + 2, :] for tp in range(npair)]
            hs_one = hs_big[:, full - 1, :] if full % 2 else None
            hs_rag = None
            if rag:
                hs_rag = hsp.tile([rag, HID], F8, tag="hsr")
                nc.scalar.dma_start(
                    out=hs_rag, in_=hs8[full * 128 : full * 128 + rag, :]
                )
            ones_sb = consts.tile([1, HN], BF16)
            nc.vector.memset(ones_sb, 1.0)
            ident = consts.tile([HN, HN], BF16)
            make_identity(nc, ident)
            pT_sb = consts.tile([128, ntile, HN], F8)  # holds q = p~ - 1
            denc = consts.tile([HN, nch], F32)
            den_sb = consts.tile([HN, 1], F32)
            ctx_sb = consts.tile([HN, HID], BF16)

            # ---- phase 1: scores -> exp -> transposed p~ ----------------
            # s_ps[hn, s] accumulates QSCALE * (q~ . hs) over 16 DoubleRow
            # fp8 matmuls (256 contraction rows each), plus one k=1 bf16
            # matmul adding the per-position mask bias via ones[hn] x mb[s].
            # Per-chunk chain [scores, exp, q, transposes] measures fastest
            # on HW: batching or staggering the transposes later delays the
            # early pT tiles phase 2 needs, and folding the -1 into the
            # transpose PSUM group adds PE work where PE is the scarce
            # resource (f32 transposes run 2 cyc/row vs bf16's 1).
            with (
                tc.tile_pool(name="sps", bufs=2, space="PSUM") as sps,
                tc.tile_pool(name="tps", bufs=2, space="PSUM") as tps,
                tc.tile_pool(name="pcb", bufs=2) as pcb,
            ):
                off = 0
                for ch, w in enumerate(widths):
                    s_ps = sps.tile([128, 512], F32, tag="s")
                    for t in range(NK // 2):
                        nc.tensor.matmul(
                            s_ps[0:HN, 0:w],
                            qe_sb[:, 2 * t : 2 * t + 2, :],
                            hsT_t[ch][:, 2 * t : 2 * t + 2, :],
                            start=(t == 0),
                            stop=False,
                            perf_mode=mybir.MatmulPerfMode.DoubleRow,
                        )
                    nc.tensor.matmul(
                        s_ps[0:HN, 0:w],
                        ones_sb,
                        mb_sb[:, off : off + w],
                        start=False,
                        stop=True,
                        skip_group_check=True,
                    )
                    p_blk = pcb.tile([HN, 512], BF16, tag="p")
                    nc.scalar.activation(
                        out=p_blk[:, 0:w],
                        in_=s_ps[0:HN, 0:w],
                        func=mybir.ActivationFunctionType.Exp,
                        scale=EXPSCALE,
                        accum_out=denc[:, ch : ch + 1],
                    )
                    q_blk = pcb.tile([HN, 512], BF16, tag="q")
                    nc.vector.tensor_scalar_add(
                        out=q_blk[:, 0:w], in0=p_blk[:, 0:w], scalar1=-1.0
                    )
                    nblk = (w + 127) // 128
                    for j in range(nblk):
                        bw = min(128, w - j * 128)
                        t_ps = tps.tile([128, 1024], BF16, tag="t")
                        nc.tensor.transpose(
                            t_ps[0:bw, 0:HN],
                            q_blk[:, j * 128 : j * 128 + bw],
                            ident,
                        )
                        nc.vector.tensor_copy(
                            out=pT_sb[0:bw, off // 128 + j, :],
                            in_=t_ps[0:bw, 0:HN],
                        )
                    off += w
                nc.vector.tensor_reduce(
                    out=den_sb,
                    in_=denc,
                    axis=mybir.AxisListType.X,
                    op=mybir.AluOpType.add,
                )

            # ---- phase 2: ctx = p~ @ hs over the prefetched hs tiles ----
            with tc.tile_pool(name="cps", bufs=1, space="PSUM") as cps:
                ctx_ps = [
                    cps.tile([128, 512], F32, tag=f"c{cg}", name=f"ctx{cg}")
                    for cg in range(NCG)
                ]
                # DoubleRow pairs: each [128, 2, HID] tile holds two
                # 128-token blocks side by side in the free dim, matching
                # pT_sb's [:, 2tp:2tp+2, :] stationary slice. An odd full
                # tile and/or the ragged tail tile (rag rows, contraction
                # K=rag) run as plain fp8 matmul sets at the end.
                last_is_pair = full % 2 == 0 and rag == 0
                for tp in range(npair):
                    for cg in range(NCG):
                        nc.tensor.matmul(
                            ctx_ps[cg][0:HN, :],
                            pT_sb[:, 2 * tp : 2 * tp + 2, :],
                            hs_pair[tp][:, :, cg * 512 : (cg + 1) * 512],
                            start=(tp == 0),
                            stop=(last_is_pair and tp == npair - 1),
                            perf_mode=mybir.MatmulPerfMode.DoubleRow,
                        )
                if full % 2:
                    for cg in range(NCG):
                        nc.tensor.matmul(
                            ctx_ps[cg][0:HN, :],
                            pT_sb[:, full - 1, :],
                            hs_one[:, cg * 512 : (cg + 1) * 512],
                            start=False,
                            stop=(rag == 0),
                            skip_group_check=True,
                        )
                if rag:
                    for cg in range(NCG):
                        nc.tensor.matmul(
                            ctx_ps[cg][0:HN, :],
                            pT_sb[0:rag, ntile - 1, :],
                            hs_rag[:, cg * 512 : (cg + 1) * 512],
                            start=False,
                            stop=True,
                            skip_group_check=True,
                        )
                # Drain bank PAIRS per engine (ACT: 0+1, 4+5; DVE: 2+3, 6+7)
                # so each 1024-col output region has a single writer — a DMA
                # spanning two engines' writes raced intermittently on HW.
                # Pair DMAs split across the SP and ACT rings (both idle by
                # now) to halve the per-ring DGE spacing at the tail.
                for pair in range(NCG // 2):
                    eng_act = pair % 2 == 0
                    for cg in (2 * pair, 2 * pair + 1):
                        sl = slice(cg * 512, (cg + 1) * 512)
                        if eng_act:
                            nc.scalar.copy(
                                out=ctx_sb[:, sl], in_=ctx_ps[cg][0:HN, :]
                            )
                        else:
                            nc.vector.tensor_copy(
                                out=ctx_sb[:, sl], in_=ctx_ps[cg][0:HN, :]
                            )
                    osl = slice(2 * pair * 512, (2 * pair + 2) * 512)
                    ring = nc.sync if eng_act else nc.scalar
                    ring.dma_start(out=ctx_out[:, osl], in_=ctx_sb[:, osl])
                nc.sync.dma_start(out=den_out, in_=den_sb)

    nc.compile()
    return nc


def _get_module(cap=DEF_CAP, reps=1):
    if (cap, reps) not in _cache:
        _cache[(cap, reps)] = _build_module(cap, reps)
    return _cache[(cap, reps)]


def _prep_in_maps(hs, mask, ms, Wq, Wk, Wv, Wo):
    """Compact away masked tokens, shard into 8 per-core input maps."""
    # qe[h*8+n, :] = (Q[n, h*64:(h+1)*64] @ Wk[h*64:(h+1)*64, :]) * QSCALE
    Q = ms @ Wq.T  # [slots, BD]
    Qh = Q.reshape(SLOTS, HEADS, HD)
    Wkh = Wk.reshape(HEADS, HD, HID)
    qe = np.einsum("nhd,hdi->hni", Qh, Wkh, optimize=True).reshape(HN, HID)
    qe = (qe * QSCALE).astype(np.float32)
    # pre-tile for a contiguous DMA: row ki holds [ko, hn] blocks
    qe8_host = np.ascontiguousarray(
        qe.T.reshape(NK, 128, HN).transpose(1, 0, 2).reshape(128, NK * HN)
    ).astype(npf8)

    kept = [np.flatnonzero(mask[b] != 0) for b in range(B)]
    need = max((len(k) + HALVES - 1) // HALVES for k in kept)
    cap = _round_cap(need)

    in_maps = []
    sbars = []  # per-batch exact token sum S, added to ctx on host
    for b in range(B):
        idx = kept[b]
        hs_keep = hs[b, idx, :]  # [T, HID] f32
        sbars.append(hs_keep.sum(axis=0, dtype=np.float64).astype(np.float32))
        T = len(idx)
        t0 = (T + 1) // 2
        for g, gsl in enumerate((slice(0, t0), slice(t0, T))):
            part = hs_keep[gsl]
            t = part.shape[0]
            hs8 = np.zeros((cap, HID), npf8)
            hs8[:t] = part.astype(npf8)
            hsT = np.zeros((HID, cap), npf8)
            hsT[:, :t] = part.T.astype(npf8)
            # pre-tile per score-chunk: [128, NK*w] blocks, ko-major per row
            hsT8 = np.concatenate(
                [
                    np.ascontiguousarray(
                        hsT[:, o : o + w]
                        .reshape(NK, 128, w)
                        .transpose(1, 0, 2)
                        .reshape(128, NK * w)
                    )
                    for o, w in zip(np.cumsum([0] + _chunks(cap)[:-1]), _chunks(cap))
                ],
                axis=1,
            )
            mb = np.full((1, cap), np.float32(MASK_NEG), npbf16)
            mb[0, :t] = npbf16(0.0)
            in_maps.append(
                {"hsT8": hsT8, "hs8": hs8, "qe8": qe8_host, "mbT": mb}
            )
    return in_maps, cap, sbars


def _host_finish(res, Wv, Wo, sbars):
    """Combine per-core ctx/den partials and apply the tiny projections.
    Device ships sum_s q*hs8 (q = p~-1); the uniform-attention part S is
    added here exactly."""
    Wvh = Wv.reshape(HEADS, HD, HID)  # [h, d, i]
    y = np.empty((B, SLOTS, HID), np.float32)
    for b in range(B):
        r0 = res[HALVES * b]
        r1 = res[HALVES * b + 1]
        numer = (
            r0["ctx_out"].astype(np.float32)
            + r1["ctx_out"].astype(np.float32)
            + sbars[b][None, :]
        )
        den = r0["den_out"] + r1["den_out"]  # [HN, 1]
        ctx = (numer / den).reshape(HEADS, SLOTS, HID)  # [h, n, i]
        z = np.einsum("hni,hdi->nhd", ctx, Wvh, optimize=True)  # [n, h, d]
        y[b] = z.reshape(SLOTS, BD) @ Wo.T
    return y


def _timing_setup(inputs_np, reps=1, shared=None):
    """Shared scaffolding for the dev-only timing helpers: a compiled
    sharded executable, device-resident inputs, and a fresh-donated-zeros
    factory. Mirrors bass2jax.run_bass_via_pjrt's multi-core path.

    `shared` (a dict) carries the device-resident input buffers across
    calls with different `reps`: physical DRAM placement of the ~75 MB of
    inputs varies per allocation and measurably changes DMA throughput, so
    a reps_lo/reps_hi slope taken over two independent allocations is
    contaminated — both executables must read the SAME buffers.
    """
    import jax
    from jax.experimental.shard_map import shard_map
    from jax.sharding import Mesh, NamedSharding, PartitionSpec

    import concourse.mybir as mybir_
    from concourse import bass2jax

    in_maps, cap, _sb = _prep_in_maps(
        np.asarray(inputs_np["hidden_states"], np.float32),
        np.asarray(inputs_np["attention_mask"]),
        np.asarray(inputs_np["memory_slots"], np.float32),
        np.asarray(inputs_np["Wq"], np.float32),
        np.asarray(inputs_np["Wk"], np.float32),
        np.asarray(inputs_np["Wv"], np.float32),
        np.asarray(inputs_np["Wo"], np.float32),
    )
    nc = _get_module(cap, reps)
    bass2jax.install_neuronx_cc_hook()

    in_names, out_names, out_avals, zero_outs = [], [], [], []
    has_partition = False
    for alloc in nc.m.functions[0].allocations:
        if not isinstance(alloc, mybir_.MemoryLocationSet):
            continue
        name = alloc.memorylocations[0].name
        if alloc.kind == "ExternalInput":
            if name == "partition_id":
                has_partition = True
                continue
            in_names.append(name)
        elif alloc.kind == "ExternalOutput":
            out_names.append(name)
            shape = tuple(alloc.tensor_shape)
            dtype = mybir_.dt.np(alloc.dtype)
            out_avals.append(jax.core.ShapedArray(shape, dtype))
            zero_outs.append(np.zeros(shape, dtype))
    n_params = len(in_names)
    n_outs = len(out_avals)
    # Operand order must match run_bass_via_pjrt: inputs, donated output
    # zeros, then partition-id LAST (neuronx_cc_hook checks operands[:-1]
    # are jit parameters 0..N-1).
    all_names = in_names + out_names + (["partition_id"] if has_partition else [])

    def _body(*args):
        operands = list(args)
        if has_partition:
            operands.append(bass2jax.partition_id_tensor())
        outs = bass2jax._bass_exec_p.bind(
            *operands,
            out_avals=tuple(out_avals),
            in_names=tuple(all_names),
            out_names=tuple(out_names),
            lowering_input_output_aliases=(),
            sim_require_finite=True,
            sim_require_nnan=True,
            nc=nc,
        )
        return tuple(outs)

    devices = jax.devices()[:N_CORES]
    mesh = Mesh(np.asarray(devices), ("core",))
    spec = PartitionSpec("core")
    sharded = jax.jit(
        shard_map(
            _body,
            mesh=mesh,
            in_specs=(spec,) * (n_params + n_outs),
            out_specs=(spec,) * n_outs,
            check_rep=False,
        ),
        donate_argnums=tuple(range(n_params, n_params + n_outs)),
        keep_unused=True,
    )
    sh = NamedSharding(mesh, spec)
    if shared is not None and "dev_in" in shared:
        assert shared["in_names"] == in_names
        dev_in = shared["dev_in"]
    else:
        concat_in = [
            np.concatenate(
                [np.asarray(in_maps[c][nm]) for c in range(N_CORES)], axis=0
            )
            for nm in in_names
        ]
        dev_in = [jax.device_put(a, sh) for a in concat_in]
        jax.block_until_ready(dev_in)
        if shared is not None:
            shared["dev_in"] = dev_in
            shared["in_names"] = in_names

    def make_dz():
        zeros = [np.zeros((N_CORES * z.shape[0], *z.shape[1:]), z.dtype)
                 for z in zero_outs]
        dz = [jax.device_put(z, sh) for z in zeros]
        jax.block_until_ready(dz)
        return dz

    return sharded, dev_in, make_dz, jax


def time_device(inputs_np, reps=8):
    """Blocking round-trip per-exec wall times (includes full axon RPC
    latency each call)."""
    import time

    sharded, dev_in, make_dz, jax = _timing_setup(inputs_np)
    out = sharded(*dev_in, *make_dz())  # warmup
    jax.block_until_ready(out)
    times = []
    for _ in range(reps):
        dz = make_dz()
        t0 = time.perf_counter()
        out = sharded(*dev_in, *dz)
        jax.block_until_ready(out)
        times.append(time.perf_counter() - t0)
    return times


def time_device_pipelined(inputs_np, depth=64, rounds=2):
    """Per-exec wall time with `depth` executes in flight: amortizes the
    axon round-trip latency, giving the closest wall-clock estimate of
    per-execution device cost available on this tunneled setup."""
    import time

    sharded, dev_in, make_dz, jax = _timing_setup(inputs_np)
    out = sharded(*dev_in, *make_dz())  # warmup
    jax.block_until_ready(out)
    per_exec = []
    for _ in range(rounds):
        dzs = [make_dz() for _ in range(depth)]
        t0 = time.perf_counter()
        outs = [sharded(*dev_in, *dzs[i]) for i in range(depth)]
        jax.block_until_ready(outs)
        per_exec.append((time.perf_counter() - t0) / depth)
    return per_exec


def time_device_marginal(inputs_np, reps_lo=4, reps_hi=132, depth=48, rounds=5):
    """True per-execution device time, measured as the marginal wall-clock
    cost of adding kernel repetitions INSIDE the NEFF.

    The axon tunnel costs ~2.5 ms per execute RPC even for a no-op NEFF
    (measured: a 2-instruction kernel times identically to this one under
    time_device_pipelined), so wall-clock per-execute says nothing about
    the kernel. Instead we compile the same kernel body emitted reps_lo
    and reps_hi times back-to-back in one NEFF (every rep redoes all DMAs
    and matmuls, writing identical bytes to the same outputs), measure
    per-execute wall time for each, and report the slope
        (T_hi - T_lo) / (reps_hi - reps_lo)
    — the RPC/dispatch overhead cancels in the difference, leaving the
    device-side cost of one full kernel execution.

    Noise control: executes are CHAINED (each call's donated output
    buffers are the previous call's outputs) so the steady state moves no
    host data at all, and lo/hi rounds are interleaved with the median of
    per-round slopes reported, cancelling slow drift in tunnel load.
    The rep spread is WIDE (4 vs 132): each compiled executable's DRAM
    placement perturbs its wall time by up to ~0.5 ms, so the slope needs
    a large denominator (0.5 ms / 128 reps = ~4 us noise) — narrow spreads
    (e.g. 36/68) produced 25-70 us readings for identical kernels. Both
    executables also read the SAME device-resident input buffers (see
    _timing_setup's `shared`) so input placement cancels exactly.
    """
    import time

    shared = {}

    def _make_runner(reps):
        sharded, dev_in, make_dz, jax = _timing_setup(
            inputs_np, reps=reps, shared=shared
        )
        outs = [sharded(*dev_in, *make_dz())]  # warmup (compiles)
        jax.block_until_ready(outs[0])

        def run(depth=depth):
            t0 = time.perf_counter()
            for _ in range(depth):
                outs[0] = sharded(*dev_in, *outs[0])
            jax.block_until_ready(outs[0])
            return (time.perf_counter() - t0) / depth

        return run, jax

    run_lo, jax = _make_runner(reps_lo)
    run_hi, _ = _make_runner(reps_hi)
    run_lo(4)  # settle
    run_hi(4)
    slopes, lows, highs = [], [], []
    for _ in range(rounds):
        t_lo = run_lo()
        t_hi = run_hi()
        lows.append(t_lo)
        highs.append(t_hi)
        slopes.append((t_hi - t_lo) / (reps_hi - reps_lo))
    per_exec = float(np.median(slopes))
    return per_exec, float(np.median(lows)), float(np.median(highs))


def kernel(hidden_states, attention_mask, memory_slots, Wq, Wk, Wv, Wo):
    global LAST_RESULT
    hs = np.asarray(hidden_states, dtype=np.float32)
    mask = np.asarray(attention_mask)
    ms = np.asarray(memory_slots, dtype=np.float32)
    Wq = np.asarray(Wq, dtype=np.float32)
    Wk = np.asarray(Wk, dtype=np.float32)
    Wv = np.asarray(Wv, dtype=np.float32)
    Wo = np.asarray(Wo, dtype=np.float32)

    in_maps, cap, sbars = _prep_in_maps(hs, mask, ms, Wq, Wk, Wv, Wo)
    nc = _get_module(cap)

    kwargs = {}
    if TRACE:
        kwargs = {"trace": True}
        if TRACE_CORES is not None:
            kwargs["trace_cores"] = TRACE_CORES
    res = run_bass_kernel_spmd(nc, in_maps, core_ids=list(range(N_CORES)), **kwargs)
    LAST_RESULT = res

    y = _host_finish(res.results, Wv, Wo, sbars)
    return np.ascontiguousarray(y.astype(np.float32))



# revision 34
# speedup vs baseline: 1.0235x; 1.0235x over previous
"""Trainium2 Bass kernel: memory-slot cross-attention (nn_LocalConstructorMulti).

Algebraic restructuring vs the reference:
    scores[b,h,n,s] = (Q[n,h,:] . K[b,s,h,:]) / 8
                    = hs[b,s,:] . qe[h*8+n,:]        with qe = fold(Wk, Wq, ms)
    out[b,n,h,:]    = Wv_h @ ctx[b,h*8+n,:]          with ctx = attn-weighted
                                                     sum of hidden states
    y[b,n,:]        = sum_h Wo_h (Wv_h ctx_hn)

So the device only computes, per (batch, seq-half) core:
    Phase 1: s[hn, s]  = qe8.T @ hsT8   (fp8, DoubleRow)   [64, cap]
             p~[hn, s] = exp(s * EXPSCALE)                  (unnormalized)
    Phase 2: dev[hn,:] = (p~ - 1) @ hs8 (fp8, DoubleRow)   [64, 4096]
             den[hn]   = sum_s p~                          (ACT accum_out)
with ctx = S + dev assembled on host (S = exact f32 token sum). The tiny
Wv/Wo projections and the 1/den normalization also happen on host (linear
ops commute with the attention sum; den differs per (h,n) so normalization
must precede the Wo mix, after summing the two seq-halves).

This removes the big K/V projections entirely and the kernel becomes
DMA-bound on reading hs once in fp8e4m3 s-major plus once transposed
(fp8 is accurate enough for scores because quantization noise averages
over the 4096-long contraction, and for ctx because only the small
fluctuation q = p~ - 1 multiplies it — see the hs8 declaration comment).

Masked tokens contribute exactly zero (p~ = exp(-1e9 * EXPSCALE) = 0),
so the host drops them before sharding: only unmasked tokens are shipped,
padded to a per-core capacity rounded up to a multiple of 32 (1056 for
this mask; the trailing cap % 128 rows run as a ragged partial-width
tile). Padding rows have hs = 0, so they score exactly 0, giving p~ = 1
and q = 0: zero ctx contribution; den is inflated by exactly the pad
count, which the host subtracts.

Sharding: 8 cores = 4 batches x 2 halves of each batch's unmasked-token
list. Host sums the halves' unnormalized ctx/den, normalizes, applies
Wv/Wo.
"""

import sys

if "/opt/trn_rl_repo" not in sys.path:
    sys.path.insert(0, "/opt/trn_rl_repo")

import ml_dtypes
import numpy as np

import concourse.bass as bass  # noqa: F401  (AP helpers)
import concourse.mybir as mybir
import concourse.tile as tile
from concourse import bacc
from concourse.bass_utils import run_bass_kernel_spmd
from concourse.masks import make_identity

BF16 = mybir.dt.bfloat16
F32 = mybir.dt.float32
F8 = mybir.dt.float8e4
npbf16 = ml_dtypes.bfloat16
npf8 = ml_dtypes.float8_e4m3

B, S, HID = 4, 4096, 4096
SLOTS, HEADS, BD = 8, 8, 512
HD = BD // HEADS  # 64
N_CORES = 8
HALVES = 2
HN = HEADS * SLOTS  # 64 (head, slot) pairs
NK = HID // 128  # 32 contraction k-tiles
NCG = HID // 512  # 8 ctx col groups (PSUM banks)
QSCALE = 512.0  # qe pre-scale so fp8 values sit in the normal range
SCALE = 1.0 / float(np.sqrt(HD))
EXPSCALE = 1.0 / (QSCALE / SCALE)  # undo QSCALE, apply 1/sqrt(HD)
MASK_NEG = -1.0e9  # after *EXPSCALE -> -244k -> exp -> 0
# Per-core token capacity (multiple of 32): roundup(max tokens per core, 32)
# computed at prep time; DEF_CAP only sizes the default module for warmup.
# The trailing cap%128 rows form a ragged tile handled at partial width, so
# padding waste is at most 31 tokens (vs 128-aligned capacities).
DEF_CAP = 1056
# No-op filler matmuls (zero stationary, +0 accumulate) appended to each
# ctx tile burst: they keep the Tensor engine continuously busy between hs
# tile arrivals so it holds its full 2.4 GHz p-state (any idle gap drops
# it to 1.2 GHz and the real matmuls take 2x longer).
PE_FILL = 0

# test.py can flip this to capture an NTFF profile; harness never touches it.
TRACE = False
TRACE_CORES = None
LAST_RESULT = None

_cache = {}


def _chunks(cap):
    """Score chunk widths: 512s then an optional remainder (PSUM bank = 512)."""
    out = [512] * (cap // 512)
    if cap % 512:
        out.append(cap % 512)
    assert sum(out) == cap
    return out


def _round_cap(need):
    """Token capacity for a per-core token count: multiple of 32, >= 256."""
    return max(256, ((need + 31) // 32) * 32)


def _build_module(cap, reps=1):
    """Emit + compile the single-core Bass module (same NEFF on all cores).

    reps > 1 emits the identical kernel body `reps` times back-to-back in
    one NEFF (each rep redoes every DMA and matmul, writing the same
    outputs with identical data). Used only for timing: the marginal cost
    per rep measures true device execution throughput with the multi-ms
    axon per-execute RPC overhead amortized away.
    """
    assert cap % 32 == 0
    full = cap // 128  # full 128-row token tiles
    rag = cap % 128  # trailing ragged tile rows (multiple of 32)
    ntile = full + (1 if rag else 0)
    widths = _chunks(cap)
    nch = len(widths)

    nc = bacc.Bacc("TRN2", target_bir_lowering=False, debug=False, num_devices=N_CORES)

    # hsT8 is pre-tiled on host: chunk ch occupies cols [NK*off, NK*(off+w))
    # with per-partition-contiguous [ko, col] blocks, so each chunk DMA is a
    # single long run per partition (short runs pay a 2x DMA penalty).
    hsT8 = nc.dram_tensor("hsT8", [128, NK * cap], F8, kind="ExternalInput").ap()
    # ctx phase reads fp8 hidden states and contracts q = p~ - 1 against
    # them: attention here is near-uniform (p~ in [0.91, 1.10]), so
    # sum_s p~*hs = S + sum_s q*hs8 + eps, where S (the exact token sum) is
    # added on HOST in f32 and eps (~0.06% RMS) collects the q-fp8 and
    # hs-fp8 quantization cross terms. With q in fp8 too, both ctx matmul
    # operands are fp8 -> DoubleRow contracts two 128-token tiles per
    # matmul at 0.5 cycles/row.
    hs8 = nc.dram_tensor("hs8", [cap, HID], F8, kind="ExternalInput").ap()
    qe8 = nc.dram_tensor("qe8", [128, NK * HN], F8, kind="ExternalInput").ap()
    ctx_out = nc.dram_tensor("ctx_out", [HN, HID], BF16, kind="ExternalOutput").ap()
    den_out = nc.dram_tensor("den_out", [HN, 1], F32, kind="ExternalOutput").ap()

    with tile.TileContext(nc) as tc:
      for _rep in range(reps):
        with (
            tc.tile_pool(name="consts", bufs=1) as consts,
            tc.tile_pool(name="hsp", bufs=9) as hsp,
        ):
            # ---- resident tensors -----------------------------------------
            # DMA ring split: sync (SP) carries qe/mb + the hsT chunks
            # (they gate the long phase-1 chain), scalar (ACT) streams the
            # phase-2 hs8 tiles concurrently — on HW the two HWDGE rings
            # together sustain more bandwidth than one (the cost-model sim
            # says otherwise; believe the hardware). den_out's DMA is
            # emitted at the very END: it waits on all of phase 1, and
            # sitting between the hsT and hs8 DMAs on one ring it stalled
            # phase 2's stream (~4 us).
            hsT_big = consts.tile([128, NK * cap], F8, name="hsTbig")
            hsT_t = []
            off = 0
            for ch, w in enumerate(widths):
                hsT_t.append(
                    hsT_big[:, NK * off : NK * (off + w)].rearrange(
                        "p (a b) -> p a b", a=NK
                    )
                )
                off += w
            qe_sb = consts.tile([128, NK, HN], F8)
            # qe/mb ride at the head of the scalar ring (ahead of hs8) so
            # the sync ring's first issue is the single merged hsT transfer
            # (one 33 KB-per-partition contiguous run, one DGE issue).
            nc.scalar.dma_start(
                out=qe_sb, in_=qe8.rearrange("p (a b) -> p a b", a=NK)
            )
            nc.sync.dma_start(out=hsT_big, in_=hsT8)
            # phase-2 hs row-tiles: one resident [128, full, HID] tile
            # streamed by TWO merged DMAs (plus the ragged tail) on the
            # scalar ring. Descriptor efficiency is identical to per-tile
            # DMAs (4 KB runs per partition either way), but the DGE issue
            # cost on the ring head drops from ~10x667 ns to ~3x667 ns.
            npair = full // 2
            hs_big = consts.tile([128, full, HID], F8, name="hsbig")
            h1 = (full + 1) // 2
            nc.scalar.dma_start(
                out=hs_big[:, 0:h1, :],
                in_=hs8[0 : h1 * 128, :].rearrange("(a p) f -> p a f", p=128),
            )
            if full > h1:
                nc.scalar.dma_start(
                    out=hs_big[:, h1:full, :],
                    in_=hs8[h1 * 128 : full * 128, :].rearrange(
                        "(a p) f -> p a f", p=128
                    ),
                )
            hs_pair = [hs_big[:, 2 * tp : 2 * tp + 2, :] for tp in range(npair)]
            hs_one = hs_big[:, full - 1, :] if full % 2 else None
            hs_rag = None
            if rag:
                hs_rag = hsp.tile([rag, HID], F8, tag="hsr")
                nc.scalar.dma_start(
                    out=hs_rag, in_=hs8[full * 128 : full * 128 + rag, :]
                )
            ones_sb = consts.tile([1, HN], BF16)
            nc.vector.memset(ones_sb, 1.0)
            ident = consts.tile([HN, HN], BF16)
            make_identity(nc, ident)
            pT_sb = consts.tile([128, ntile, HN], F8)  # holds q = p~ - 1
            denc = consts.tile([HN, nch], F32)
            den_sb = consts.tile([HN, 1], F32)
            ctx_sb = consts.tile([HN, HID], BF16)

            # ---- phase 1: scores -> exp -> transposed p~ ----------------
            # s_ps[hn, s] accumulates QSCALE * (q~ . hs) over 16 DoubleRow
            # fp8 matmuls (256 contraction rows each), plus one k=1 bf16
            # matmul adding the per-position mask bias via ones[hn] x mb[s].
            # Per-chunk chain [scores, exp, q, transposes] measures fastest
            # on HW: batching or staggering the transposes later delays the
            # early pT tiles phase 2 needs, and folding the -1 into the
            # transpose PSUM group adds PE work where PE is the scarce
            # resource (f32 transposes run 2 cyc/row vs bf16's 1).
            with (
                tc.tile_pool(name="sps", bufs=2, space="PSUM") as sps,
                tc.tile_pool(name="tps", bufs=2, space="PSUM") as tps,
                tc.tile_pool(name="pcb", bufs=2) as pcb,
            ):
                off = 0
                for ch, w in enumerate(widths):
                    s_ps = sps.tile([128, 512], F32, tag="s")
                    for t in range(NK // 2):
                        nc.tensor.matmul(
                            s_ps[0:HN, 0:w],
                            qe_sb[:, 2 * t : 2 * t + 2, :],
                            hsT_t[ch][:, 2 * t : 2 * t + 2, :],
                            start=(t == 0),
                            stop=(t == NK // 2 - 1),
                            perf_mode=mybir.MatmulPerfMode.DoubleRow,
                        )
                    p_blk = pcb.tile([HN, 512], BF16, tag="p")
                    nc.scalar.activation(
                        out=p_blk[:, 0:w],
                        in_=s_ps[0:HN, 0:w],
                        func=mybir.ActivationFunctionType.Exp,
                        scale=EXPSCALE,
                        accum_out=denc[:, ch : ch + 1],
                    )
                    q_blk = pcb.tile([HN, 512], BF16, tag="q")
                    nc.vector.tensor_scalar_add(
                        out=q_blk[:, 0:w], in0=p_blk[:, 0:w], scalar1=-1.0
                    )
                    nblk = (w + 127) // 128
                    for j in range(nblk):
                        bw = min(128, w - j * 128)
                        t_ps = tps.tile([128, 1024], BF16, tag="t")
                        nc.tensor.transpose(
                            t_ps[0:bw, 0:HN],
                            q_blk[:, j * 128 : j * 128 + bw],
                            ident,
                        )
                        nc.vector.tensor_copy(
                            out=pT_sb[0:bw, off // 128 + j, :],
                            in_=t_ps[0:bw, 0:HN],
                        )
                    off += w
                nc.vector.tensor_reduce(
                    out=den_sb,
                    in_=denc,
                    axis=mybir.AxisListType.X,
                    op=mybir.AluOpType.add,
                )

            # ---- phase 2: ctx = p~ @ hs over the prefetched hs tiles ----
            with tc.tile_pool(name="cps", bufs=1, space="PSUM") as cps:
                ctx_ps = [
                    cps.tile([128, 512], F32, tag=f"c{cg}", name=f"ctx{cg}")
                    for cg in range(NCG)
                ]
                # DoubleRow pairs: each [128, 2, HID] tile holds two
                # 128-token blocks side by side in the free dim, matching
                # pT_sb's [:, 2tp:2tp+2, :] stationary slice. An odd full
                # tile and/or the ragged tail tile (rag rows, contraction
                # K=rag) run as plain fp8 matmul sets at the end.
                last_is_pair = full % 2 == 0 and rag == 0
                for tp in range(npair):
                    for cg in range(NCG):
                        nc.tensor.matmul(
                            ctx_ps[cg][0:HN, :],
                            pT_sb[:, 2 * tp : 2 * tp + 2, :],
                            hs_pair[tp][:, :, cg * 512 : (cg + 1) * 512],
                            start=(tp == 0),
                            stop=(last_is_pair and tp == npair - 1),
                            perf_mode=mybir.MatmulPerfMode.DoubleRow,
                        )
                if full % 2:
                    for cg in range(NCG):
                        nc.tensor.matmul(
                            ctx_ps[cg][0:HN, :],
                            pT_sb[:, full - 1, :],
                            hs_one[:, cg * 512 : (cg + 1) * 512],
                            start=False,
                            stop=(rag == 0),
                            skip_group_check=True,
                        )
                if rag:
                    for cg in range(NCG):
                        nc.tensor.matmul(
                            ctx_ps[cg][0:HN, :],
                            pT_sb[0:rag, ntile - 1, :],
                            hs_rag[:, cg * 512 : (cg + 1) * 512],
                            start=False,
                            stop=True,
                            skip_group_check=True,
                        )
                # Drain bank PAIRS per engine (ACT: 0+1, 4+5; DVE: 2+3, 6+7)
                # so each 1024-col output region has a single writer — a DMA
                # spanning two engines' writes raced intermittently on HW.
                # Pair DMAs split across the SP and ACT rings (both idle by
                # now) to halve the per-ring DGE spacing at the tail.
                for pair in range(NCG // 2):
                    eng_act = pair % 2 == 0
                    for cg in (2 * pair, 2 * pair + 1):
                        sl = slice(cg * 512, (cg + 1) * 512)
                        if eng_act:
                            nc.scalar.copy(
                                out=ctx_sb[:, sl], in_=ctx_ps[cg][0:HN, :]
                            )
                        else:
                            nc.vector.tensor_copy(
                                out=ctx_sb[:, sl], in_=ctx_ps[cg][0:HN, :]
                            )
                    osl = slice(2 * pair * 512, (2 * pair + 2) * 512)
                    ring = nc.sync if eng_act else nc.scalar
                    ring.dma_start(out=ctx_out[:, osl], in_=ctx_sb[:, osl])
                nc.sync.dma_start(out=den_out, in_=den_sb)

    nc.compile()
    return nc


def _get_module(cap=DEF_CAP, reps=1):
    if (cap, reps) not in _cache:
        _cache[(cap, reps)] = _build_module(cap, reps)
    return _cache[(cap, reps)]


def _prep_in_maps(hs, mask, ms, Wq, Wk, Wv, Wo):
    """Compact away masked tokens, shard into 8 per-core input maps."""
    # qe[h*8+n, :] = (Q[n, h*64:(h+1)*64] @ Wk[h*64:(h+1)*64, :]) * QSCALE
    Q = ms @ Wq.T  # [slots, BD]
    Qh = Q.reshape(SLOTS, HEADS, HD)
    Wkh = Wk.reshape(HEADS, HD, HID)
    qe = np.einsum("nhd,hdi->hni", Qh, Wkh, optimize=True).reshape(HN, HID)
    qe = (qe * QSCALE).astype(np.float32)
    # pre-tile for a contiguous DMA: row ki holds [ko, hn] blocks
    qe8_host = np.ascontiguousarray(
        qe.T.reshape(NK, 128, HN).transpose(1, 0, 2).reshape(128, NK * HN)
    ).astype(npf8)

    kept = [np.flatnonzero(mask[b] != 0) for b in range(B)]
    need = max((len(k) + HALVES - 1) // HALVES for k in kept)
    cap = _round_cap(need)

    in_maps = []
    pads = []  # per-core zero-pad token count; each pad adds exp(0)=1 to den
    sbars = []  # per-batch exact token sum S, added to ctx on host
    for b in range(B):
        idx = kept[b]
        hs_keep = hs[b, idx, :]  # [T, HID] f32
        sbars.append(hs_keep.sum(axis=0, dtype=np.float64).astype(np.float32))
        T = len(idx)
        t0 = (T + 1) // 2
        for g, gsl in enumerate((slice(0, t0), slice(t0, T))):
            part = hs_keep[gsl]
            t = part.shape[0]
            hs8 = np.zeros((cap, HID), npf8)
            hs8[:t] = part.astype(npf8)
            hsT = np.zeros((HID, cap), npf8)
            hsT[:, :t] = part.T.astype(npf8)
            # pre-tile per score-chunk: [128, NK*w] blocks, ko-major per row
            hsT8 = np.concatenate(
                [
                    np.ascontiguousarray(
                        hsT[:, o : o + w]
                        .reshape(NK, 128, w)
                        .transpose(1, 0, 2)
                        .reshape(128, NK * w)
                    )
                    for o, w in zip(np.cumsum([0] + _chunks(cap)[:-1]), _chunks(cap))
                ],
                axis=1,
            )
            pads.append(cap - t)
            in_maps.append({"hsT8": hsT8, "hs8": hs8, "qe8": qe8_host})
    return in_maps, cap, sbars, pads


def _host_finish(res, Wv, Wo, sbars, pads):
    """Combine per-core ctx/den partials and apply the tiny projections.
    Device ships sum_s q*hs8 (q = p~-1); the uniform-attention part S is
    added here exactly."""
    Wvh = Wv.reshape(HEADS, HD, HID)  # [h, d, i]
    y = np.empty((B, SLOTS, HID), np.float32)
    for b in range(B):
        r0 = res[HALVES * b]
        r1 = res[HALVES * b + 1]
        numer = (
            r0["ctx_out"].astype(np.float32)
            + r1["ctx_out"].astype(np.float32)
            + sbars[b][None, :]
        )
        den = (
            r0["den_out"] + r1["den_out"]
            - (pads[HALVES * b] + pads[HALVES * b + 1])
        )  # [HN, 1]; zero-pad columns score 0 -> p~ = 1 each
        ctx = (numer / den).reshape(HEADS, SLOTS, HID)  # [h, n, i]
        z = np.einsum("hni,hdi->nhd", ctx, Wvh, optimize=True)  # [n, h, d]
        y[b] = z.reshape(SLOTS, BD) @ Wo.T
    return y


def _timing_setup(inputs_np, reps=1, shared=None):
    """Shared scaffolding for the dev-only timing helpers: a compiled
    sharded executable, device-resident inputs, and a fresh-donated-zeros
    factory. Mirrors bass2jax.run_bass_via_pjrt's multi-core path.

    `shared` (a dict) carries the device-resident input buffers across
    calls with different `reps`: physical DRAM placement of the ~75 MB of
    inputs varies per allocation and measurably changes DMA throughput, so
    a reps_lo/reps_hi slope taken over two independent allocations is
    contaminated — both executables must read the SAME buffers.
    """
    import jax
    from jax.experimental.shard_map import shard_map
    from jax.sharding import Mesh, NamedSharding, PartitionSpec

    import concourse.mybir as mybir_
    from concourse import bass2jax

    in_maps, cap, _sb, _pads = _prep_in_maps(
        np.asarray(inputs_np["hidden_states"], np.float32),
        np.asarray(inputs_np["attention_mask"]),
        np.asarray(inputs_np["memory_slots"], np.float32),
        np.asarray(inputs_np["Wq"], np.float32),
        np.asarray(inputs_np["Wk"], np.float32),
        np.asarray(inputs_np["Wv"], np.float32),
        np.asarray(inputs_np["Wo"], np.float32),
    )
    nc = _get_module(cap, reps)
    bass2jax.install_neuronx_cc_hook()

    in_names, out_names, out_avals, zero_outs = [], [], [], []
    has_partition = False
    for alloc in nc.m.functions[0].allocations:
        if not isinstance(alloc, mybir_.MemoryLocationSet):
            continue
        name = alloc.memorylocations[0].name
        if alloc.kind == "ExternalInput":
            if name == "partition_id":
                has_partition = True
                continue
            in_names.append(name)
        elif alloc.kind == "ExternalOutput":
            out_names.append(name)
            shape = tuple(alloc.tensor_shape)
            dtype = mybir_.dt.np(alloc.dtype)
            out_avals.append(jax.core.ShapedArray(shape, dtype))
            zero_outs.append(np.zeros(shape, dtype))
    n_params = len(in_names)
    n_outs = len(out_avals)
    # Operand order must match run_bass_via_pjrt: inputs, donated output
    # zeros, then partition-id LAST (neuronx_cc_hook checks operands[:-1]
    # are jit parameters 0..N-1).
    all_names = in_names + out_names + (["partition_id"] if has_partition else [])

    def _body(*args):
        operands = list(args)
        if has_partition:
            operands.append(bass2jax.partition_id_tensor())
        outs = bass2jax._bass_exec_p.bind(
            *operands,
            out_avals=tuple(out_avals),
            in_names=tuple(all_names),
            out_names=tuple(out_names),
            lowering_input_output_aliases=(),
            sim_require_finite=True,
            sim_require_nnan=True,
            nc=nc,
        )
        return tuple(outs)

    devices = jax.devices()[:N_CORES]
    mesh = Mesh(np.asarray(devices), ("core",))
    spec = PartitionSpec("core")
    sharded = jax.jit(
        shard_map(
            _body,
            mesh=mesh,
            in_specs=(spec,) * (n_params + n_outs),
            out_specs=(spec,) * n_outs,
            check_rep=False,
        ),
        donate_argnums=tuple(range(n_params, n_params + n_outs)),
        keep_unused=True,
    )
    sh = NamedSharding(mesh, spec)
    if shared is not None and "dev_in" in shared:
        assert shared["in_names"] == in_names
        dev_in = shared["dev_in"]
    else:
        concat_in = [
            np.concatenate(
                [np.asarray(in_maps[c][nm]) for c in range(N_CORES)], axis=0
            )
            for nm in in_names
        ]
        dev_in = [jax.device_put(a, sh) for a in concat_in]
        jax.block_until_ready(dev_in)
        if shared is not None:
            shared["dev_in"] = dev_in
            shared["in_names"] = in_names

    def make_dz():
        zeros = [np.zeros((N_CORES * z.shape[0], *z.shape[1:]), z.dtype)
                 for z in zero_outs]
        dz = [jax.device_put(z, sh) for z in zeros]
        jax.block_until_ready(dz)
        return dz

    return sharded, dev_in, make_dz, jax


def time_device(inputs_np, reps=8):
    """Blocking round-trip per-exec wall times (includes full axon RPC
    latency each call)."""
    import time

    sharded, dev_in, make_dz, jax = _timing_setup(inputs_np)
    out = sharded(*dev_in, *make_dz())  # warmup
    jax.block_until_ready(out)
    times = []
    for _ in range(reps):
        dz = make_dz()
        t0 = time.perf_counter()
        out = sharded(*dev_in, *dz)
        jax.block_until_ready(out)
        times.append(time.perf_counter() - t0)
    return times


def time_device_pipelined(inputs_np, depth=64, rounds=2):
    """Per-exec wall time with `depth` executes in flight: amortizes the
    axon round-trip latency, giving the closest wall-clock estimate of
    per-execution device cost available on this tunneled setup."""
    import time

    sharded, dev_in, make_dz, jax = _timing_setup(inputs_np)
    out = sharded(*dev_in, *make_dz())  # warmup
    jax.block_until_ready(out)
    per_exec = []
    for _ in range(rounds):
        dzs = [make_dz() for _ in range(depth)]
        t0 = time.perf_counter()
        outs = [sharded(*dev_in, *dzs[i]) for i in range(depth)]
        jax.block_until_ready(outs)
        per_exec.append((time.perf_counter() - t0) / depth)
    return per_exec


def time_device_marginal(inputs_np, reps_lo=4, reps_hi=132, depth=48, rounds=5):
    """True per-execution device time, measured as the marginal wall-clock
    cost of adding kernel repetitions INSIDE the NEFF.

    The axon tunnel costs ~2.5 ms per execute RPC even for a no-op NEFF
    (measured: a 2-instruction kernel times identically to this one under
    time_device_pipelined), so wall-clock per-execute says nothing about
    the kernel. Instead we compile the same kernel body emitted reps_lo
    and reps_hi times back-to-back in one NEFF (every rep redoes all DMAs
    and matmuls, writing identical bytes to the same outputs), measure
    per-execute wall time for each, and report the slope
        (T_hi - T_lo) / (reps_hi - reps_lo)
    — the RPC/dispatch overhead cancels in the difference, leaving the
    device-side cost of one full kernel execution.

    Noise control: executes are CHAINED (each call's donated output
    buffers are the previous call's outputs) so the steady state moves no
    host data at all, and lo/hi rounds are interleaved with the median of
    per-round slopes reported, cancelling slow drift in tunnel load.
    The rep spread is WIDE (4 vs 132): each compiled executable's DRAM
    placement perturbs its wall time by up to ~0.5 ms, so the slope needs
    a large denominator (0.5 ms / 128 reps = ~4 us noise) — narrow spreads
    (e.g. 36/68) produced 25-70 us readings for identical kernels. Both
    executables also read the SAME device-resident input buffers (see
    _timing_setup's `shared`) so input placement cancels exactly.
    """
    import time

    shared = {}

    def _make_runner(reps):
        sharded, dev_in, make_dz, jax = _timing_setup(
            inputs_np, reps=reps, shared=shared
        )
        outs = [sharded(*dev_in, *make_dz())]  # warmup (compiles)
        jax.block_until_ready(outs[0])

        def run(depth=depth):
            t0 = time.perf_counter()
            for _ in range(depth):
                outs[0] = sharded(*dev_in, *outs[0])
            jax.block_until_ready(outs[0])
            return (time.perf_counter() - t0) / depth

        return run, jax

    run_lo, jax = _make_runner(reps_lo)
    run_hi, _ = _make_runner(reps_hi)
    run_lo(4)  # settle
    run_hi(4)
    slopes, lows, highs = [], [], []
    for _ in range(rounds):
        t_lo = run_lo()
        t_hi = run_hi()
        lows.append(t_lo)
        highs.append(t_hi)
        slopes.append((t_hi - t_lo) / (reps_hi - reps_lo))
    per_exec = float(np.median(slopes))
    return per_exec, float(np.median(lows)), float(np.median(highs))


def kernel(hidden_states, attention_mask, memory_slots, Wq, Wk, Wv, Wo):
    global LAST_RESULT
    hs = np.asarray(hidden_states, dtype=np.float32)
    mask = np.asarray(attention_mask)
    ms = np.asarray(memory_slots, dtype=np.float32)
    Wq = np.asarray(Wq, dtype=np.float32)
    Wk = np.asarray(Wk, dtype=np.float32)
    Wv = np.asarray(Wv, dtype=np.float32)
    Wo = np.asarray(Wo, dtype=np.float32)

    in_maps, cap, sbars, pads = _prep_in_maps(hs, mask, ms, Wq, Wk, Wv, Wo)
    nc = _get_module(cap)

    kwargs = {}
    if TRACE:
        kwargs = {"trace": True}
        if TRACE_CORES is not None:
            kwargs["trace_cores"] = TRACE_CORES
    res = run_bass_kernel_spmd(nc, in_maps, core_ids=list(range(N_CORES)), **kwargs)
    LAST_RESULT = res

    y = _host_finish(res.results, Wv, Wo, sbars, pads)
    return np.ascontiguousarray(y.astype(np.float32))

